# revision 4
# baseline (speedup 1.0000x reference)
"""Trainium2 Bass kernel for nn_CrossAttention_47502338294587 — v3.

Math: the reference cross-attention has a single KV position broadcast over
all T query positions.  Softmax over a row of identical logits is uniform,
so attention output == v for every query, and the whole module collapses to

    out[b, t, :] = (visual_features[b] @ Wv + bv) @ Wp + bp      (for all t)

independent of x / Wq / Wk / t.

Architecture notes (measured on this runtime):
 - NRT collective_compute costs ~55us wall for a 2KB AllGather here, and
   remote_dma* ucode is not available — so NO cross-core exchange is viable.
   Without an exchange, every core must read all of Wv (out[:, ci] =
   vf @ Wv @ Wp[:, ci]); Wv is replicated, Wp is column-sharded.
 - Compute dtype is bf16 (input HBM bytes halved, single-pass PE matmuls
   instead of the fp32 dual pass); accumulation and biases stay fp32.
   rel tolerance for this problem family (2e-2) is the bf16-compute regime.
 - The T axis of the output is mathematically constant, so the device
   emits one [B, CSH] row shard per core; the host broadcasts over T
   (pure data movement, no host arithmetic).

Per-core structure:
  mm1:  vv[b, :] = vf @ Wv  (+bv)     16 matmuls [4,512], moving=Wv chunks
  tr:   vv^T chunks via PE transpose  (mm2 contracts over the vv index)
  mm2:  row[b, ci] = vv @ Wp[:, ci] (+bp[ci])   8 matmuls, moving=Wp chunk
"""

import os
import sys

import numpy as np

for _p in ("/opt/trn_rl_repo",):
    if _p not in sys.path and os.path.isdir(_p):
        sys.path.insert(0, _p)

import ml_dtypes

BF16 = ml_dtypes.bfloat16

B, T, C = 4, 1024, 1024
N_CORES = 8
CSH = C // N_CORES  # 128, C-shard per core
KC = C // 128  # 8 contraction chunks

_BUILT = None


def build_nc():
    """Build + compile the Bass program (one NeuronCore's SPMD body)."""
    import concourse.bass as bass
    import concourse.mybir as mybir
    import concourse.tile as tile
    from concourse import bacc
    from concourse.bass import ts

    f32 = mybir.dt.float32
    bf16 = mybir.dt.bfloat16
    nc = bacc.Bacc("TRN2", target_bir_lowering=False, debug=False)

    # host pre-packs into exact SBUF layouts (pure layout prep):
    #   wv_p[p, k*C + n]   = Wv[k*128 + p, n]        (full Wv, bf16)
    #   wp_p[p, k*CSH + c] = Wp[k*128 + p, ci_c]     (column shard, bf16)
    #   vft_p[p, k*B + b]  = vf[b, k*128 + p]        (bf16)
    wv_p = nc.dram_tensor("wv_p", [128, KC * C], bf16, kind="ExternalInput")
    wp_p = nc.dram_tensor("wp_p", [128, KC * CSH], bf16, kind="ExternalInput")
    vft_p = nc.dram_tensor("vft_p", [128, KC * B], bf16, kind="ExternalInput")
    # bvt[p, k] = bv[k*128 + p] — added per-partition at the vvt copy stage
    bvt = nc.dram_tensor("bvt", [128, KC], f32, kind="ExternalInput")
    bp4 = nc.dram_tensor("bp4", [B, CSH], f32, kind="ExternalInput")
    # out[b, c_local] = row shard; host re-assembles + broadcasts over T
    out = nc.dram_tensor("out", [B, CSH], f32, kind="ExternalOutput")

    with tile.TileContext(nc) as tc:
        with tc.tile_pool(name="sb", bufs=1) as sb:
            wv_t = [sb.tile([128, C], bf16, name=f"wv{k}", tag=f"wv{k}") for k in range(KC)]
            wp_t = sb.tile([128, KC, CSH], bf16, tag="wp_t")
            vft_t = sb.tile([128, KC, B], bf16, tag="vft")
            bvt_t = sb.tile([128, KC], f32, tag="bvt")
            bp4_t = sb.tile([B, CSH], f32, tag="bp4")
            vv_sb = sb.tile([B, C], bf16, tag="vv_sb")
            vvt_t = sb.tile([128, KC, B], bf16, tag="vvt")
            ident_t = sb.tile([B, B], bf16, tag="ident")
            row_sb = sb.tile([B, CSH], f32, tag="row_sb")

            # identity for PE transpose: 1.0 on the diagonal band
            nc.gpsimd.memset(ident_t[:], 1.0)
            nc.gpsimd.affine_select(
                out=ident_t[:], in_=ident_t[:], compare_op=mybir.AluOpType.is_ge,
                fill=0.0, base=0, pattern=[[1, B]], channel_multiplier=-1,
            )
            nc.gpsimd.affine_select(
                out=ident_t[:], in_=ident_t[:], compare_op=mybir.AluOpType.is_ge,
                fill=0.0, base=0, pattern=[[-1, B]], channel_multiplier=1,
            )

            # ---- DMA in. mm1 is jointly bound by the PE rate and the
            # replicated-wv HBM read (8 cores x 2MB), so the wv chunks lead
            # on both HWDGE queues with nothing slow in front of them.
            # wv chunk 0 is split in half so the first matmul starts off a
            # 128KB transfer; vft (gates the first LDWEIGHTS) heads the
            # scalar queue; wp and the tiny biases trail — they are not
            # needed until ~8us later.
            nc.sync.dma_start(wv_t[0][:, 0:512], wv_p[:, 0:512])
            nc.scalar.dma_start(wv_t[1][:, 0:512], wv_p[:, C : C + 512])
            nc.sync.dma_start(wv_t[0][:, 512:C], wv_p[:, 512:C])
            nc.scalar.dma_start(vft_t[:], vft_p.rearrange("p (k b) -> p k b", b=B))
            nc.scalar.dma_start(wv_t[1][:, 512:C], wv_p[:, C + 512 : 2 * C])
            for k in range(2, KC):
                eng = nc.sync if k % 2 == 0 else nc.scalar
                eng.dma_start(wv_t[k][:], wv_p[:, ts(k, C)])
            nc.scalar.dma_start(wp_t[:], wp_p.rearrange("p (k c) -> p k c", c=CSH))
            nc.sync.dma_start(bvt_t[:], bvt[:, :])
            nc.sync.dma_start(bp4_t[:], bp4[:, :])

            # ---- mm1: vv[b, n] = sum_k vf[b, k-blk] Wv[k-blk, n] ----------
            with tc.tile_pool(name="pv", bufs=2, space="PSUM") as pv:
                psum_vv = [pv.tile([B, 512], f32, name=f"pvv{h}", tag=f"pvv{h}") for h in range(2)]
                for h in range(2):
                    for k in range(KC):
                        nc.tensor.matmul(
                            psum_vv[h][:],
                            vft_t[:, k, :],
                            wv_t[k][:, ts(h, 512)],
                            start=(k == 0),
                            stop=(k == KC - 1),
                        )
                    # PSUM->SBUF drains on two engines in parallel (the bias
                    # is added later, per-partition, during the vvt copies)
                    if h == 0:
                        nc.vector.tensor_copy(vv_sb[0:B, ts(h, 512)], psum_vv[h][:])
                    else:
                        nc.scalar.activation(
                            vv_sb[0:B, ts(h, 512)],
                            psum_vv[h][:],
                            mybir.ActivationFunctionType.Copy,
                        )

            # ---- transpose vv -> vv^T chunks [128, B] -----------------------
            with tc.tile_pool(name="pt", bufs=4, space="PSUM") as pt:
                def bvt_rep(k):
                    ap = bvt_t[:, k : k + 1]
                    return bass.AP(
                        ap.tensor, ap.offset, [list(ap.ap[0]), [0, B], list(ap.ap[1])]
                    )

                for k in range(KC):
                    psum_vvt = pt.tile([128, B], bf16, tag="pvt")
                    nc.tensor.transpose(
                        psum_vvt[:], vv_sb[0:B, ts(k, 128)], ident_t[0:B, 0:B]
                    )
                    nc.vector.tensor_add(
                        vvt_t[:, k, :].rearrange("p b -> p b ()"),
                        psum_vvt[:].rearrange("p b -> p b ()"),
                        bvt_rep(k).rearrange("p b q -> p b q"),
                    )

            # ---- mm2: row[b, ci] = sum_g vv[b, g-blk] Wp[g-blk, ci] (+bp) --
            # mm2 in two column halves: the first half's bias add and output
            # DMA overlap the second half's matmuls; the two 1KB output DMAs
            # ride different queues so their completion receipts overlap.
            with tc.tile_pool(name="pr", bufs=2, space="PSUM") as pr:
                psum_row = [
                    pr.tile([B, CSH // 2], f32, name=f"prow{q}", tag=f"pr{q}")
                    for q in range(2)
                ]
                for q, eng in ((0, nc.sync), (1, nc.scalar)):
                    for g in range(KC):
                        nc.tensor.matmul(
                            psum_row[q][:],
                            vvt_t[:, g, :],
                            wp_t[:, g, ts(q, CSH // 2)],
                            start=(g == 0),
                            stop=(g == KC - 1),
                        )
                    nc.vector.tensor_add(
                        row_sb[:, ts(q, CSH // 2)], psum_row[q][:], bp4_t[:, ts(q, CSH // 2)]
                    )
                    eng.dma_start(out[:, ts(q, CSH // 2)], row_sb[:, ts(q, CSH // 2)])

    nc.compile()
    return nc


def _get_built():
    global _BUILT
    if _BUILT is None:
        _BUILT = build_nc()
    return _BUILT


def make_in_maps(inputs):
    vf = np.asarray(inputs["visual_features"], np.float32)
    wv = np.asarray(inputs["Wv"], np.float32)
    wp = np.asarray(inputs["Wp"], np.float32)
    bv = np.asarray(inputs["bv"], np.float32)
    bp = np.asarray(inputs["bp"], np.float32)
    # wv_p[p, k*C + n] = Wv[k*128 + p, n]
    wv_p = np.ascontiguousarray(
        wv.reshape(KC, 128, C).transpose(1, 0, 2).reshape(128, KC * C)
    ).astype(BF16)
    vft_p = np.ascontiguousarray(
        vf.T.reshape(KC, 128, B).transpose(1, 0, 2).reshape(128, KC * B)
    ).astype(BF16)
    # bvt[p, k] = bv[k*128 + p]
    bvt = np.ascontiguousarray(bv.reshape(KC, 128).T.astype(np.float32))
    maps = []
    for i in range(N_CORES):
        ci = slice(i * CSH, (i + 1) * CSH)
        wp_pk = np.ascontiguousarray(
            wp[:, ci].reshape(KC, 128, CSH).transpose(1, 0, 2).reshape(128, KC * CSH)
        ).astype(BF16)
        bp4 = np.ascontiguousarray(np.broadcast_to(bp[None, ci], (B, CSH)))
        maps.append(
            {"wv_p": wv_p, "wp_p": wp_pk, "vft_p": vft_p, "bvt": bvt, "bp4": bp4}
        )
    return maps


def run(inputs, trace=False, **kw):
    from concourse.bass_utils import run_bass_kernel_spmd

    nc = _get_built()
    res = run_bass_kernel_spmd(
        nc,
        make_in_maps(inputs),
        core_ids=list(range(N_CORES)),
        trace=trace,
        **kw,
    )
    row = np.empty((B, C), np.float32)
    for i, r in enumerate(res.results):
        row[:, i * CSH : (i + 1) * CSH] = r["out"]
    full = np.empty((B, T, C), np.float32)
    full[:] = row[:, None, :]
    return full, res


def kernel(**inputs) -> np.ndarray:
    full, _ = run(inputs, trace=False)
    return full
